# revision 32
# baseline (speedup 1.0000x reference)
"""Trainium2 Bass kernel for single-query gated cross-attention (DAttention).

Reference computation (per batch b, single query token at `pos`):
    q   = x[:, pos] @ Wq.T, scaled, split into 8 heads of 64
    kv  = context @ Wkv.T ; k, v = split(kv)
    dots = q @ k.T + attn_bias ; attn = softmax(mask(dots))
    out = (attn @ v) * sigmoid(x[:, pos] @ Wg.T + bg) @ Wo.T + bo

Algebraic structure: with a single query token the full K/V projections
(69 GFLOP) are unnecessary:
    dots[b,h,j] = sum_c context[b,j,c] * qk[b,h,c],  qk = q_scaled @ Wk_h
    attn-weighted V = (sum_j attn[b,h,j] * context[b,j,c]) @ Wv_h.T
The device computes dots (fp8e3 context stream x bf16 folded queries),
attn = exp(dots) * ebias (ebias = exp(bias)*mask precomputed on host, so
no bias add, no -1e30 masking, no max-subtraction on device), the
normalization sums (ones-matmul), and the attention-weighted context sum.
Normalization and the tiny O(batch) Wv/gating/Wo folds run on host.

Perf notes (measured on this hw):
  - PE streams 1 column/cycle at 2.4GHz for all dtypes; fp8e4/e5
    DoubleRow would be 2-4x but its 3-mantissa quantization blows the
    2e-2 budget (measured 2.4-3.8e-2 end to end). fp8-E3M4 it is, so the
    floor is two context passes through the PE (2 x 16384 cyc/batch)
    plus cheap transposes (~102 cyc each).
  - DMA sustains ~334GB/s on the HWDGE queue only when each descriptor
    row moves 2KB+; context layouts are pre-blocked on host so every
    transfer is [128 rows x 2-4KB contiguous].
  - HWDGE trigger lanes (8) are assigned round-robin in SCHEDULED order
    and carry ring-credit waits, so ALL input DMAs are emitted first and
    wait-free on the Sync queue in exact PE consumption order (ctxT b0,
    ctxT b1, ctxn b0, ctxn b1); only the 4 tail output DMAs (Act queue)
    come later. A DMA-crossbar attn transpose (dma_start_transpose) was
    tried twice and reverted (88-91us): its compute-gated triggers land
    between input triggers in the lane sequence, so late ctx inputs
    ring-credit-wait on transposes that wait on dots that wait on ctx.
    Splitting ctxT across both queues was also tried (54.2us): the Act
    ring is much slower (~44GB/s) and its chunks gate the dots.
  - attn head-major -> token-major runs on the PE as PAIR-transposes:
    odd j-blocks' dots/exp/mult pipeline sits at partitions 32-39 (via
    matmul tile_position=(0,32)), so attnT holds an even j-block at rows
    0-7 and its odd partner at rows 32-39 over the same columns, and one
    [40,128] transpose against eye40 emits TWO token-tiles (16 weight
    loads per batch instead of 32, ~1.3us PE saved). Transposes trail a
    full j-pair behind the dots so the exp/mult chain never stalls.
  - warmup matmuls during the initial DMA fill ramp the PE p-state.

Measured (8-core SPMD, NTFF): ~52.0us (52.5-52.8 with single-tile
transposes) vs 56.8us baseline; rel err 9.31e-3. Timeline: ~7.2us NEFF
start barrier + table loads before the first DMA trigger, dots+transpose
front paced by the 2.1MB/batch ctxT stream at ~334GB/s, sums+both wsums
at full PE rate (~221ns per 512-col matmul), ~5us copy + drain tail.

Sharding: data-parallel over batch (16 batches / 8 cores = 2 per core).
No collectives; host gathers the per-core [2, 9, 512] result (8 rows of
weighted context sums + 1 row of packed normalization sums).
"""

import numpy as np
import ml_dtypes

import concourse.bass as bass
import concourse.bacc as bacc
import concourse.tile as tile
import concourse.mybir as mybir
from concourse.bass_utils import run_bass_kernel_spmd

BF16 = mybir.dt.bfloat16
F32 = mybir.dt.float32
FP8 = mybir.dt.float8e3
NP_BF16 = ml_dtypes.bfloat16
NP_FP8 = ml_dtypes.float8_e3m4

N_CORES = 8
B = 16
N = 4096
DIM = 512
HEADS = 8
DIM_HEAD = 64
INNER = HEADS * DIM_HEAD
SCALE = DIM_HEAD ** -0.5
BPC = B // N_CORES          # batches per core (2)
KC = DIM // 128             # dots contraction chunks (4)
NJ = 8                      # dots j-blocks of 512 tokens
NT = N // 128               # token tiles of 128 (32)
WARM = 3                    # PE warmup matmuls during initial DMA fill
JBYTES = 128 * KC * 512     # bytes of one ctxT j-block
TBYTES = 128 * 512          # bytes of one ctxn token tile

# ctxT DMA chunks in j-blocks: batch 0 leads with two single blocks so the
# first dots matmul starts ~1.3us earlier.
CTXT_CHUNKS = [[1, 1, 2, 2, 2], [2, 2, 2, 2]]
# ctxn DMA groups in token tiles: small leading groups so the first wsum
# matmul starts right as the stream turns over.
CTXN_GROUPS = [4, 4, 8, 8, 8]

# consts column map (bf16): [qkT: k*16 + b*8 + h][64: ones][65-168: eye104]
CW = KC * 16 + 1 + 104


def _build_nc():
    nc = bacc.Bacc("TRN2", target_bir_lowering=False, debug=False,
                   num_devices=N_CORES)

    consts_d = nc.dram_tensor("consts", [128, CW], BF16, kind="ExternalInput")
    ebias_d = nc.dram_tensor("ebias", [4 * HEADS, BPC, NJ // 4, 512], BF16,
                             kind="ExternalInput")
    ctxT_d = nc.dram_tensor("ctxT", [BPC, NJ * JBYTES], FP8,
                            kind="ExternalInput")
    ctxn_d = nc.dram_tensor("ctxn", [BPC, NT * TBYTES], FP8,
                            kind="ExternalInput")
    out_d = nc.dram_tensor("out9", [BPC, 9, 512], F32, kind="ExternalOutput")

    with tile.TileContext(nc) as tc:
        with (
            tc.tile_pool(name="const", bufs=1) as const_pool,
            tc.tile_pool(name="ctx", bufs=1) as ctx_pool,
            tc.tile_pool(name="attn", bufs=1) as attn_pool,
            tc.tile_pool(name="esb", bufs=3) as e_pool,
            tc.tile_pool(name="osb", bufs=1) as o_pool,
            tc.tile_pool(name="pd", bufs=2, space="PSUM") as pd_pool,
            tc.tile_pool(name="ptr", bufs=2, space="PSUM") as ptr_pool,
            tc.tile_pool(name="pacc", bufs=1, space="PSUM") as pacc_pool,
            tc.tile_pool(name="psum", bufs=1, space="PSUM") as psums_pool,
        ):
            # ---- input DMAs, wait-free, in PE consumption order; the
            # first ctxT chunk leads so the first dots starts earliest ----
            consts_sb = const_pool.tile([128, CW], BF16, tag="consts")
            # j%4 == r ebias at partition rows 32r..32r+7 (each j-block's
            # dots pipeline runs at its own 32-aligned partition group so
            # one PE transpose emits four token-tiles at once)
            ebias_sb = const_pool.tile([104, BPC, NJ // 4, 512], BF16,
                                       tag="ebias")
            warm_sb = const_pool.tile([128, 512], FP8, tag="warm")
            nc.vector.memset(warm_sb[:], 0.125)

            # ctxT: per-chunk partition-major contiguous blocks
            ctxT_tiles = [{} for _ in range(BPC)]   # j -> (tile, slot)
            first = True
            for b in range(BPC):
                j0 = 0
                for ci, cn in enumerate(CTXT_CHUNKS[b]):
                    t = ctx_pool.tile([128, cn * KC, 512], FP8,
                                      tag=f"ctxT{b}_{ci}", name=f"ctxT{b}_{ci}")
                    src = ctxT_d[b, j0 * JBYTES:(j0 + cn) * JBYTES]
                    nc.sync.dma_start(
                        out=t[:], in_=src.rearrange("(p x) -> p x", p=128))
                    for s in range(cn):
                        ctxT_tiles[b][j0 + s] = (t, s)
                    j0 += cn
                    if first:
                        # small consts ride right behind the leading chunk
                        nc.sync.dma_start(out=consts_sb[:], in_=consts_d[:])
                        for r in range(4):
                            nc.sync.dma_start(
                                out=ebias_sb[32 * r:32 * r + HEADS],
                                in_=ebias_d[r * HEADS:(r + 1) * HEADS])
                        first = False
            # ctxn: per-group contiguous [128, gs, 512] blocks
            ctxn_tiles = [{} for _ in range(BPC)]   # t -> (tile, slot)
            for b in range(BPC):
                t0 = 0
                for gi, gs in enumerate(CTXN_GROUPS):
                    t = ctx_pool.tile([128, gs, 512], FP8,
                                      tag=f"ctxn{b}_{gi}", name=f"ctxn{b}_{gi}")
                    src = ctxn_d[b, t0 * TBYTES:(t0 + gs) * TBYTES]
                    nc.sync.dma_start(
                        out=t[:], in_=src.rearrange("(p x) -> p x", p=128))
                    for s in range(gs):
                        ctxn_tiles[b][t0 + s] = (t, s)
                    t0 += gs

            attnT = [attn_pool.tile([104, N // 4], BF16, tag=f"attnT{b}",
                                    name=f"attnT{b}") for b in range(BPC)]
            attn_nat = [attn_pool.tile([128, NT, HEADS], BF16, tag=f"an{b}",
                                       name=f"an{b}") for b in range(BPC)]
            pacc = [pacc_pool.tile([HEADS, DIM], F32, tag=f"pa{b}",
                                   name=f"pa{b}") for b in range(BPC)]
            psums = [psums_pool.tile([1, NT * HEADS], F32, tag=f"ps{b}",
                                     name=f"ps{b}") for b in range(BPC)]

            # ---- PE warmup during the DMA fill (p-state ramp) ----
            for i in range(WARM):
                pw = pd_pool.tile([HEADS, 512], F32, tag="pd")
                nc.tensor.matmul(pw[:], lhsT=warm_sb[:, :HEADS], rhs=warm_sb[:],
                                 start=True, stop=True)

            def dots_block(b, j):
                tl, s = ctxT_tiles[b][j]
                po = 32 * (j % 4)
                pd = pd_pool.tile([104, 512], F32, tag="pd")
                for k in range(KC):
                    nc.tensor.matmul(
                        pd[po:po + HEADS, :],
                        lhsT=consts_sb[:, k * 16 + b * 8:k * 16 + b * 8 + 8],
                        rhs=tl[:, s * KC + k, :],
                        start=(k == 0),
                        stop=(k == KC - 1),
                        tile_position=(0, po),
                    )
                e_sb = e_pool.tile([104, 512], BF16, tag="e")
                nc.scalar.activation(
                    e_sb[po:po + HEADS, :], pd[po:po + HEADS, :],
                    mybir.ActivationFunctionType.Exp)
                nc.vector.tensor_tensor(
                    out=attnT[b][po:po + HEADS,
                                 (j // 4) * 512:(j // 4 + 1) * 512],
                    in0=e_sb[po:po + HEADS, :],
                    in1=ebias_sb[po:po + HEADS, b, j // 4, :],
                    op=mybir.AluOpType.mult,
                )

            def transpose_block(b, jq):
                ptr = ptr_pool.tile([128, 4, 104], BF16, tag="ptr")
                for tt in range(4):
                    nc.tensor.transpose(
                        ptr[:, tt, :],
                        attnT[b][:, jq * 512 + tt * 128:jq * 512 + (tt + 1) * 128],
                        consts_sb[0:104, KC * 16 + 1:KC * 16 + 105],
                    )
                for r in range(4):
                    nc.vector.tensor_copy(
                        attn_nat[b][:, 16 * jq + 4 * r:16 * jq + 4 * r + 4, :],
                        ptr[:, :, 32 * r:32 * r + HEADS])

            # dots chase the ctxT stream; quad-transposes trail a full
            # j-quad behind so the exp/mult chain never stalls the PE.
            for j in range(NJ):
                dots_block(0, j)
                if j == 5:
                    transpose_block(0, 0)
            for j in range(NJ):
                dots_block(1, j)
                if j == 1:
                    transpose_block(0, 1)
                if j == 5:
                    transpose_block(1, 0)

            def batch_tail(b):
                # sums first: its copy + output DMA drain while wsum runs
                nc.tensor.matmul(
                    psums[b][:],
                    lhsT=consts_sb[:, KC * 16:KC * 16 + 1],
                    rhs=attn_nat[b][:],
                    start=True, stop=True,
                )
                outs = o_pool.tile([1, NT * HEADS], F32, tag=f"os{b}",
                                   name=f"os{b}")
                nc.vector.tensor_copy(outs[:], psums[b][:])
                nc.scalar.dma_start(out=out_d[b][8:9, 0:NT * HEADS], in_=outs[:])
                for t in range(NT):
                    tl, s = ctxn_tiles[b][t]
                    nc.tensor.matmul(
                        pacc[b][:],
                        lhsT=attn_nat[b][:, t, :],
                        rhs=tl[:, s, :],
                        start=(t == 0),
                        stop=(t == NT - 1),
                    )
                outt = o_pool.tile([HEADS, 512], F32, tag=f"o{b}", name=f"o{b}")
                nc.vector.tensor_copy(outt[:], pacc[b][:])
                nc.scalar.dma_start(out=out_d[b][0:HEADS, :], in_=outt[:])

            batch_tail(0)
            transpose_block(1, 1)
            batch_tail(1)

    nc.compile()
    return nc


_NC_CACHE = None


def _get_nc():
    global _NC_CACHE
    if _NC_CACHE is None:
        _NC_CACHE = _build_nc()
    return _NC_CACHE


def _host_prep(x, context, attn_bias, Wq, Wkv, Wg, bg, mask, context_mask, pos):
    """Fold the query-side projections and build per-core device inputs."""
    pos = int(pos)
    qx = np.asarray(x[:, pos, :], dtype=np.float32)              # [B, DIM]
    Wq = np.asarray(Wq, np.float32)
    Wkv = np.asarray(Wkv, np.float32)
    q = (qx @ Wq.T).reshape(B, HEADS, DIM_HEAD) * SCALE          # [B, 8, 64]
    Wk = Wkv[:INNER].reshape(HEADS, DIM_HEAD, DIM)               # [8, 64, DIM]
    qk = np.einsum("bhd,hdc->bhc", q, Wk)                        # [B, 8, DIM]

    # multiplicative bias: exp(bias) with the mask folded in as exact zeros
    full_mask = (np.asarray(mask, bool).reshape(B, 1, 1)
                 & np.asarray(context_mask, bool).reshape(B, 1, N))
    ebias = np.where(full_mask,
                     np.exp(np.asarray(attn_bias, np.float32).reshape(B, HEADS, N)),
                     0.0).astype(NP_BF16)                        # [B, 8, N]

    ctx_f8 = np.asarray(context, np.float32).astype(NP_FP8)      # [B, N, DIM]
    in_maps = []
    for c in range(N_CORES):
        bs = slice(c * BPC, (c + 1) * BPC)
        ctx_c = ctx_f8[bs]
        # ctxT per-DMA-chunk partition-major blocks:
        # chunk[p, j_in_chunk, k, t] = ctx[b, (j0+j)*512+t, k*128+p]
        ctxT = np.zeros((BPC, NJ * JBYTES), dtype=NP_FP8)
        for b in range(BPC):
            j0 = 0
            for cn in CTXT_CHUNKS[b]:
                blk = ctx_c[b, j0 * 512:(j0 + cn) * 512].reshape(
                    cn, 512, KC, 128).transpose(3, 0, 2, 1)
                ctxT[b, j0 * JBYTES:(j0 + cn) * JBYTES] = \
                    np.ascontiguousarray(blk).reshape(-1)
                j0 += cn
        # ctxn per-group partition-major blocks: [128, gs, 512] contiguous
        blocks = []
        t0 = 0
        for gs in CTXN_GROUPS:
            blk = ctx_c[:, t0 * 128:(t0 + gs) * 128].reshape(
                BPC, gs, 128, 512).transpose(0, 2, 1, 3)
            blocks.append(np.ascontiguousarray(blk).reshape(BPC, -1))
            t0 += gs
        ctxn = np.concatenate(blocks, axis=1)
        consts = np.zeros((128, CW), dtype=NP_BF16)
        qkc = qk[bs].astype(NP_BF16)                             # [2, 8, DIM]
        for b in range(BPC):
            for k in range(KC):
                consts[:, k * 16 + b * 8:k * 16 + b * 8 + 8] = \
                    qkc[b, :, k * 128:(k + 1) * 128].T
        consts[:, KC * 16] = NP_BF16(1.0)
        consts[0:104, KC * 16 + 1:KC * 16 + 105] = np.eye(104, dtype=NP_BF16)
        ehbj = ebias[bs].transpose(1, 0, 2).reshape(HEADS, BPC, NJ // 4, 4, 512)
        eb = np.ascontiguousarray(
            np.concatenate([ehbj[:, :, :, r, :] for r in range(4)], axis=0))
        in_maps.append({
            "consts": consts,
            "ebias": eb,
            "ctxT": ctxT,
            "ctxn": ctxn,
        })
    return in_maps


def _host_epilogue(out9, x, Wkv, Wo, bo, Wg, bg, pos):
    """out9[b, 0:8, :] = unnormalized weighted ctx sums; out9[b, 8, 0:256]
    = per-(token-tile, head) partial normalization sums."""
    pos = int(pos)
    qx = np.asarray(x[:, pos, :], dtype=np.float32)
    sums = out9[:, 8, 0:NT * HEADS].reshape(B, NT, HEADS).sum(axis=1)
    acc = out9[:, 0:HEADS, :] / sums[..., None]                  # [B, 8, DIM]
    Wv = np.asarray(Wkv, np.float32)[INNER:].reshape(HEADS, DIM_HEAD, DIM)
    out_v = np.einsum("bhc,hdc->bhd", acc, Wv).reshape(B, INNER)
    gates = qx @ np.asarray(Wg, np.float32).T + np.asarray(bg, np.float32)
    inner = out_v * (1.0 / (1.0 + np.exp(-gates)))
    out = inner @ np.asarray(Wo, np.float32).T + np.asarray(bo, np.float32)
    return out.reshape(B, 1, DIM).astype(np.float32)


def run_device(in_maps, trace=False):
    nc = _get_nc()
    return run_bass_kernel_spmd(nc, in_maps, list(range(N_CORES)), trace=trace)


def kernel(x, context, attn_bias, Wq, Wkv, Wo, bo, Wg, bg, mask, context_mask,
           pos, _trace=False, _results=None):
    in_maps = _host_prep(x, context, attn_bias, Wq, Wkv, Wg, bg,
                         mask, context_mask, pos)
    res = run_device(in_maps, trace=_trace)
    if _results is not None:
        _results.append(res)
    out9 = np.concatenate([res.results[c]["out9"] for c in range(N_CORES)],
                          axis=0).astype(np.float32)
    return _host_epilogue(out9, x, Wkv, Wo, bo, Wg, bg, pos)
